# revision 58
# baseline (speedup 1.0000x reference)
"""Single-head causal attention (B=8, T=4096, C=1024, H=64) on 8 trn2 NeuronCores.

Sharding: pure data-parallel over batch — core b computes batch element b
(no collectives needed).

Per-core algorithm (v3). All matmul operands are bf16 (1 cyc/col on the PE
vs f32r's measured ~2 cyc/col); PSUM accumulation and the softmax
normalization stay fp32.

  x pipeline (per 512-row t-slice):
    - gpsimd (SWDGE) casting DMA: x fp32 in HBM -> xa bf16 [128, 4, 1024]
      in SBUF (cast happens in the DMA engines, no compute cost).
    - PE-transpose to x^T in bf16 (1 cyc/col vs fp32's ~5): 8 transposes
      per 1-bank bf16 psum tile, evacuated by one DVE copy each.
    - Projection pass 1: lhsT = [Wq|Wq] -> Q^T duplicated on both
      partition halves; pass 2: lhsT = [Wk|Wv] -> K^T (top, duplicated to
      the bottom half via SBUF-SBUF DMA) and V^T (bottom). Evacuations on
      DVE (the ACT engine is reserved for exp).
    - V' build: PE-transpose V^T -> [V | 1] per k-chunk (the ones column
      makes the A@V matmul emit softmax denominators for free).
  Attention (per 512-col q-block q):
    for k-chunk pairs (j, j+1), j <= 4q+3:
      S^T chunks [tk:128, tq:512] -> one 2-bank psum pair tile
      A^T = exp(S^T/8) in ONE [128,1024] ACT op per pair (fp32 psum ->
        bf16). No max-subtraction: scores ~ N(0,1), exp never overflows.
      diagonal chunks: skip fully-masked leading columns and multiply the
      128-col triangular window by an upper-tri mask (DVE, bf16 2x mode).
      psum_O [65, 512] += matmul(lhsT=[V|1], rhs=A^T)  (row 64 = denom)
    Output tail: PE-transpose psum_O -> [tq, 65], reciprocal of the
    denominator column, scale, DMA out.

  Scheduling: the x-pipeline/projection stream (stage A) and the
  attention stream (stage B) are interleaved at ~1us granularity: after
  each score-pair's matmuls one stage-A step is emitted before the pair's
  AV matmuls, so the PE works on projections while the ACT engine
  computes the pair's exp. PSUM: 8 banks = psA (1-bank tiles, 2 bufs:
  proj passes, V', output transposes) + scores pairs (2-bank, 2 bufs) +
  psum_O (1-bank, 2 bufs).
"""

from contextlib import ExitStack

import numpy as np

import concourse.bass as bass
import concourse.mybir as mybir
import concourse.tile as tile
from concourse import bacc
from concourse.masks import make_identity
from concourse.bass_utils import run_bass_kernel_spmd

F32 = mybir.dt.float32
BF16 = mybir.dt.bfloat16
F32R = mybir.dt.float32r

DT = BF16

B = 8
T = 4096
C = 1024
H = 64

TS = 512          # t-slice width (stage A) and q-block width (stage B)
N_CORES = 8


def build_nc(t_len: int = T, dt_c=None):
    """Build + compile the per-core Bass program for sequence length t_len."""
    if dt_c is None:
        dt_c = DT
    assert t_len % TS == 0
    n_slice = t_len // TS          # t-slices / q-blocks
    n_chunk = t_len // 128         # 128-wide k-chunks / t-tiles

    nc = bacc.Bacc(None, target_bir_lowering=False, debug=False)

    x_d = nc.dram_tensor("x", [t_len, C], F32, kind="ExternalInput")
    wq_d = nc.dram_tensor("wq", [C, H], F32, kind="ExternalInput")
    wk_d = nc.dram_tensor("wk", [C, H], F32, kind="ExternalInput")
    wv_d = nc.dram_tensor("wv", [C, H], F32, kind="ExternalInput")
    out_d = nc.dram_tensor("out", [t_len, H], F32, kind="ExternalOutput")

    with tile.TileContext(nc) as tc, ExitStack() as ctx:
        const_pool = ctx.enter_context(tc.tile_pool(name="const", bufs=1))
        res_pool = ctx.enter_context(tc.tile_pool(name="resident", bufs=1))
        xa_pool = ctx.enter_context(tc.tile_pool(name="xa", bufs=2))
        xt_pool = ctx.enter_context(tc.tile_pool(name="xt", bufs=2))
        at_pool = ctx.enter_context(tc.tile_pool(name="at", bufs=5))
        osb_pool = ctx.enter_context(tc.tile_pool(name="osb", bufs=2))
        fin_pool = ctx.enter_context(tc.tile_pool(name="fin", bufs=2))
        rec_pool = ctx.enter_context(tc.tile_pool(name="rec", bufs=2))
        ps_a = ctx.enter_context(tc.tile_pool(name="ps_a", bufs=3, space="PSUM"))
        ps_s = ctx.enter_context(tc.tile_pool(name="ps_s", bufs=2, space="PSUM"))
        ps_o = ctx.enter_context(tc.tile_pool(name="ps_o", bufs=1, space="PSUM"))

        # ---- x loads: gpsimd (SWDGE) DMAs cast fp32 -> bf16 in flight.
        # xa[s][g][p, c] = x[s*TS + 128*g + p, c]. Slices 0,1 are loaded as
        # four per-g DMAs each (fine-grained deps so the first transposes
        # start after 512 KB, not 2 MB); later slices use one DMA.
        xa_tiles: dict = {}
        xa_coarse: dict = {}

        def issue_x_load(s, gs=(0, 1, 2, 3)):
            if s not in xa_tiles:
                xa_tiles[s] = [
                    xa_pool.tile([128, C], DT, tag=f"xag{g}",
                                 name=f"xa{s}_{g}")
                    for g in range(4)
                ]
            for g in gs:
                nc.gpsimd.dma_start(
                    out=xa_tiles[s][g][:],
                    in_=x_d[s * TS + 128 * g : s * TS + 128 * (g + 1), :],
                )

        # slice 0 of x starts loading before anything else hits the Pool
        # queue (its DMA gates the first transposes)
        issue_x_load(0)

        # ---- constants ----
        identf = const_pool.tile([128, 128], F32, tag="identf")
        make_identity(nc, identf[:])
        # ident2[64+i, i] = 1 (identity content living at partitions 64:128)
        scr2 = const_pool.tile([128, H], F32, tag="scr2")
        nc.gpsimd.memset(scr2[:], 0.0)
        nc.gpsimd.affine_select(
            out=scr2[:],
            in_=scr2[:],
            compare_op=mybir.AluOpType.not_equal,
            fill=1.0,
            base=-64,
            pattern=[[-1, H]],
            channel_multiplier=1,
        )
        ident2 = const_pool.tile([128, H], dt_c, tag="ident2")
        nc.vector.tensor_copy(ident2[:], scr2[:])
        # bf16 identity for the x transposes (bf16 streams at 1 cyc/col)
        identb = const_pool.tile([128, 128], DT, tag="identb")
        nc.vector.tensor_copy(identb[:], identf[:])
        # warm the ACT exp table set (~2.7us DMA) during the initial ramp
        warm = const_pool.tile([128, 1], F32, tag="warm")
        nc.scalar.activation(
            warm[:], scr2[:, 0:1], mybir.ActivationFunctionType.Exp
        )

        # mask[x, y] = 1.0 if y >= x else 0.0 (upper-triangular window)
        ms = const_pool.tile([128, 128], F32, tag="maskscr")
        nc.gpsimd.memset(ms[:], 1.0)
        nc.gpsimd.affine_select(
            out=ms[:],
            in_=ms[:],
            compare_op=mybir.AluOpType.is_ge,
            fill=0.0,
            base=0,
            pattern=[[1, 128]],
            channel_multiplier=-1,
        )
        mask0 = const_pool.tile([128, 128], dt_c, tag="mask0")
        nc.vector.tensor_copy(mask0[:], ms[:])

        # ---- weights: gpsimd casting DMAs straight into the packed bf16
        # stationaries [Wq|Wq] and [Wk|Wv] (no fp32 staging, no DVE casts).
        # Interleaved with the first two x-slice loads on the Pool queue so
        # slice 0 starts moving first.
        wqq = const_pool.tile([128, 8, 128], dt_c, tag="wqq")
        wkv = const_pool.tile([128, 8, 128], dt_c, tag="wkv")

        def load_w(dst, half, w_d):
            nc.gpsimd.dma_start(
                out=dst[:, :, H * half : H * (half + 1)],
                in_=w_d.rearrange("(cc p) h -> p cc h", p=128),
            )

        load_w(wqq, 0, wq_d)
        load_w(wqq, 1, wq_d)
        issue_x_load(1)
        load_w(wkv, 0, wk_d)
        load_w(wkv, 1, wv_d)

        # ---- residents (bf16), ONE tile per t-slice holding Q^T | K^T |
        # V^T | V' (fewer pool tags -> fewer semaphores -> shorter
        # epilogue). The stage driver drains all of slice s before block s
        # reads it, so the coarse per-tile deps cost nothing.
        # Layout per partition: [0:512) Q^T, [512:1024) K^T (4 chunks of
        # 128), [1024:1536) V^T, [1536:1800) V' (4 groups of 66: V|1|pad).
        res = [res_pool.tile([128, 1800], dt_c, tag=f"res{s}",
                             name=f"res{s}") for s in range(n_slice)]
        qts = [r[:, 0:TS] for r in res]
        kts = [r[:, TS : 2 * TS].rearrange("p (g c) -> p g c", c=128)
               for r in res]
        vts = [r[:, 2 * TS : 3 * TS] for r in res]
        vps = [r[:, 3 * TS : 3 * TS + 264].rearrange("p (g h) -> p g h",
                                                     h=H + 2)
               for r in res]
        # (the V' ones-columns are memset inside each a_slice, keeping the
        # Pool queue clear for the early x-slice loads)

        # ---- Stage A generator: one slice = several interleavable steps.
        # Slices 0,1: PE-transpose (low latency, fine-grained x deps).
        # Slices 2+: XBAR dma_start_transpose (16x128 bf16 tiles) — the
        # transpose rides the DMA fabric, freeing ~20us of PE time; its
        # output layout is xt[p, g, cc, tau] = x^T[128cc+p, 128g+tau].
        def a_slice(s):
            xa = xa_tiles[s]
            # XBAR transposes measured SLOWER end-to-end (DMA-fabric
            # contention with the x loads); keep all transposes on the PE.
            use_xbar = False
            if use_xbar:
                xt = xt_pool.tile([128, 4, 8, 128], DT, tag="xtx",
                                  name=f"xt{s}")
                nc.sync.dma_start_transpose(xt[:], xa_coarse[s][:])
                rhs_of = lambda cc: xt[:, :, cc, :]
            else:
                xt = xt_pool.tile([128, 8, TS], DT, tag="xt", name=f"xt{s}")
                rhs_of = lambda cc: xt[:, cc, :]
            if s + 2 < n_slice:
                issue_x_load(s + 2)
            nc.gpsimd.memset(vps[s][:, :, H : H + 1], 1.0)
            if not use_xbar:
                for g in range(4):
                    pp = ps_a.tile([128, 8, 128], DT, tag="psA")
                    for cc in range(8):
                        nc.tensor.matmul(
                            pp[:, cc, :],
                            xa[g][:, 128 * cc : 128 * (cc + 1)],
                            identb[:],
                            is_transpose=True,
                            start=(cc == 0),
                            stop=(cc == 7),
                            skip_group_check=True,
                        )
                    # early slices alternate the evacuation between the ACT
                    # engine (idle until the first attention block) and DVE
                    # so the evac latency never paces the transpose stream
                    dst = xt[:, :, 128 * g : 128 * (g + 1)]
                    if s < 4 and g % 2 == 0:
                        nc.scalar.copy(dst, pp[:])
                    else:
                        nc.vector.tensor_copy(dst, pp[:])
                    yield
            else:
                yield
            # pass 1: [Wq|Wq]
            psp1 = ps_a.tile([128, TS], F32, tag="psA")
            for cc in range(4):
                nc.tensor.matmul(
                    psp1[:], wqq[:, cc, :], rhs_of(cc),
                    start=(cc == 0), stop=False,
                )
            yield
            for cc in range(4, 8):
                nc.tensor.matmul(
                    psp1[:], wqq[:, cc, :], rhs_of(cc),
                    start=False, stop=(cc == 7),
                )
            nc.vector.tensor_copy(qts[s][:], psp1[:])
            yield
            # pass 2: [Wk|Wv]
            psp2 = ps_a.tile([128, TS], F32, tag="psA")
            for cc in range(4):
                nc.tensor.matmul(
                    psp2[:], wkv[:, cc, :], rhs_of(cc),
                    start=(cc == 0), stop=False,
                )
            yield
            for cc in range(4, 8):
                nc.tensor.matmul(
                    psp2[:], wkv[:, cc, :], rhs_of(cc),
                    start=False, stop=(cc == 7),
                )
            nc.vector.tensor_copy(
                kts[s][0:64, :, :],
                psp2[0:64, :].rearrange("p (g c) -> p g c", c=128),
            )
            nc.vector.tensor_copy(vts[s][64:128, :], psp2[64:128, :])
            # duplicate K^T onto partitions 64:128: the pair's two score
            # matmuls then read disjoint SBUF partition halves (port
            # parallelism), which is what lets them pipeline tightly
            nc.sync.dma_start(kts[s][64:128, :, :], kts[s][0:64, :, :])
            yield
            # V' build: V natural [tk, 64] + ones column
            psv = ps_a.tile([128, TS], dt_c, tag="psA")
            for g in range(4):
                nc.tensor.matmul(
                    psv[:, H * g : H * (g + 1)],
                    vts[s][64:128, 128 * g : 128 * (g + 1)],
                    ident2[64:128, :],
                    is_transpose=True,
                    start=(g == 0),
                    stop=(g == 3),
                    skip_group_check=True,
                )
            nc.vector.tensor_copy(
                vps[s][:, :, 0:H],
                psv[:, 0 : 4 * H].rearrange("p (g h) -> p g h", h=H),
            )
            yield

        def a_stream():
            for s in range(n_slice):
                yield from a_slice(s)
                yield s  # slice s fully emitted

        agen = a_stream()
        a_done = [-1]

        def step_a():
            try:
                r = next(agen)
                if isinstance(r, int):
                    a_done[0] = r
            except StopIteration:
                pass

        def drain_a(upto):
            while a_done[0] < upto:
                r = next(agen)
                if isinstance(r, int):
                    a_done[0] = r

        # emit slices 0 and 1 before attention starts
        drain_a(1)

        # ---- Stage B: attention per q-block, interleaved with stage A ----
        for q in range(n_slice):
            drain_a(min(q, n_slice - 1))
            pso = ps_o.tile([H + 1, TS], F32, tag="pso")
            nj = 4 * (q + 1)

            def emit_avs(j0, ds, at):
                for u in range(2):
                    j = j0 + u
                    d = ds[u]
                    nc.tensor.matmul(
                        pso[:, d:TS],
                        vps[j // 4][:, j % 4, 0 : H + 1],
                        at[:, u, d:TS],
                        start=(j == 0),
                        stop=(j == nj - 1),
                        skip_group_check=True,
                    )

            # depth-2 software pipeline: the AV matmuls of pair k are
            # emitted after pair k+1's score matmuls and one stage-A step,
            # so pair k's exp (ACT, ~1.1us) has a ~2.3us PE-work window.
            pending = None
            for j0 in range(0, nj, 2):
                # d = first valid column of the chunk (causal): columns
                # before d are entirely masked and skipped end-to-end.
                ds = [max(0, 128 * (j0 + u) - TS * q) for u in range(2)]
                pss = ps_s.tile([128, 2, TS], F32, tag="pss")
                for u in range(2):
                    j = j0 + u
                    lo, hi = 64 * u, 64 * (u + 1)
                    nc.tensor.matmul(
                        pss[:, u, ds[u] : TS],
                        kts[j // 4][lo:hi, j % 4, :],
                        qts[q][lo:hi, ds[u] : TS],
                        start=True,
                        stop=True,
                        skip_group_check=True,
                    )
                at = at_pool.tile([128, 2, TS], dt_c)
                if ds[1] == 0:
                    nc.scalar.activation(
                        at[:], pss[:],
                        mybir.ActivationFunctionType.Exp, scale=0.125,
                    )
                else:
                    for u in range(2):
                        nc.scalar.activation(
                            at[:, u, ds[u] : TS], pss[:, u, ds[u] : TS],
                            mybir.ActivationFunctionType.Exp, scale=0.125,
                        )
                for u in range(2):
                    j = j0 + u
                    if j >= 4 * q:
                        d = ds[u]
                        # triangular window = first 128 computed columns
                        nc.vector.tensor_mul(
                            at[:, u, d : d + 128],
                            at[:, u, d : d + 128],
                            mask0[:],
                        )
                step_a()
                if pending is not None:
                    emit_avs(*pending)
                pending = (j0, ds, at)
            emit_avs(*pending)
            # O and the denominators round to bf16 before the final
            # transpose (cheaper PE transposes); normalization stays fp32
            osb = osb_pool.tile([H + 1, TS], DT)
            nc.vector.tensor_copy(osb[:], pso[:])
            # batch the 4 output transposes into one psum bank (padded to
            # H+2 per group so each bf16 group lands 4-byte aligned)
            psf = ps_a.tile([128, 4, H + 2], DT, tag="psA")
            for g in range(4):
                nc.tensor.matmul(
                    psf[:, g, 0 : H + 1],
                    osb[:, 128 * g : 128 * (g + 1)],
                    identb[0 : H + 1, 0 : H + 1],
                    is_transpose=True,
                    start=(g == 0),
                    stop=(g == 3),
                    skip_group_check=True,
                )
            rec = rec_pool.tile([128, 4, 1], F32)
            nc.vector.reciprocal(rec[:], psf[:, :, H : H + 1])
            fin = fin_pool.tile([128, 4, H], F32)
            for g in range(4):
                nc.vector.tensor_scalar_mul(
                    fin[:, g, :], psf[:, g, 0:H], rec[:, g, :]
                )
            # one DMA per q-block (4 serial issues -> 1)
            nc.sync.dma_start(
                out_d[q * TS : (q + 1) * TS, :].rearrange(
                    "(g p) h -> p g h", p=128),
                fin[:],
            )
        drain_a(n_slice - 1)

    nc.compile()
    return nc


_NC_CACHE: dict = {}


def _get_nc(t_len: int, dt_c=None):
    key = (t_len, dt_c or DT)
    if key not in _NC_CACHE:
        _NC_CACHE[key] = build_nc(t_len, dt_c)
    return _NC_CACHE[key]


def run_on_cores(nc, x_b: np.ndarray, wq, wk, wv):
    """Run the compiled program SPMD on the 8 cores; x_b is [B, t, C]."""
    in_maps = [
        {
            "x": np.ascontiguousarray(x_b[b]),
            "wq": np.ascontiguousarray(wq),
            "wk": np.ascontiguousarray(wk),
            "wv": np.ascontiguousarray(wv),
        }
        for b in range(x_b.shape[0])
    ]
    res = run_bass_kernel_spmd(nc, in_maps, list(range(len(in_maps))))
    return np.stack([res.results[b]["out"] for b in range(x_b.shape[0])])


def kernel(x, Wq, Wk, Wv):
    x = np.asarray(x, dtype=np.float32)
    Wq = np.asarray(Wq, dtype=np.float32)
    Wk = np.asarray(Wk, dtype=np.float32)
    Wv = np.asarray(Wv, dtype=np.float32)
    assert x.shape == (B, T, C), x.shape
    nc = _get_nc(T)
    return run_on_cores(nc, x, Wq, Wk, Wv)


# revision 59
# speedup vs baseline: 1.0462x; 1.0462x over previous
"""Single-head causal attention (B=8, T=4096, C=1024, H=64) on 8 trn2 NeuronCores.

Sharding: pure data-parallel over batch — core b computes batch element b
(no collectives needed).

Per-core algorithm (v3). All matmul operands are bf16 (1 cyc/col on the PE
vs f32r's measured ~2 cyc/col); PSUM accumulation and the softmax
normalization stay fp32.

  x pipeline (per 512-row t-slice):
    - gpsimd (SWDGE) casting DMA: x fp32 in HBM -> xa bf16 [128, 4, 1024]
      in SBUF (cast happens in the DMA engines, no compute cost).
    - PE-transpose to x^T in bf16 (1 cyc/col vs fp32's ~5): 8 transposes
      per 1-bank bf16 psum tile, evacuated by one DVE copy each.
    - Projection pass 1: lhsT = [Wq|Wq] -> Q^T duplicated on both
      partition halves; pass 2: lhsT = [Wk|Wv] -> K^T (top, duplicated to
      the bottom half via SBUF-SBUF DMA) and V^T (bottom). Evacuations on
      DVE (the ACT engine is reserved for exp).
    - V' build: PE-transpose V^T -> [V | 1] per k-chunk (the ones column
      makes the A@V matmul emit softmax denominators for free).
  Attention (per 512-col q-block q):
    for k-chunk pairs (j, j+1), j <= 4q+3:
      S^T chunks [tk:128, tq:512] -> one 2-bank psum pair tile
      A^T = exp(S^T/8) in ONE [128,1024] ACT op per pair (fp32 psum ->
        bf16). No max-subtraction: scores ~ N(0,1), exp never overflows.
      diagonal chunks: skip fully-masked leading columns and multiply the
      128-col triangular window by an upper-tri mask (DVE, bf16 2x mode).
      psum_O [65, 512] += matmul(lhsT=[V|1], rhs=A^T)  (row 64 = denom)
    Output tail: PE-transpose psum_O -> [tq, 65], reciprocal of the
    denominator column, scale, DMA out.

  Scheduling: the x-pipeline/projection stream (stage A) and the
  attention stream (stage B) are interleaved at ~1us granularity: after
  each score-pair's matmuls one stage-A step is emitted before the pair's
  AV matmuls, so the PE works on projections while the ACT engine
  computes the pair's exp. PSUM: 8 banks = psA (1-bank tiles, 2 bufs:
  proj passes, V', output transposes) + scores pairs (2-bank, 2 bufs) +
  psum_O (1-bank, 2 bufs).
"""

from contextlib import ExitStack

import numpy as np

import concourse.bass as bass
import concourse.mybir as mybir
import concourse.tile as tile
from concourse import bacc
from concourse.masks import make_identity
from concourse.bass_utils import run_bass_kernel_spmd

F32 = mybir.dt.float32
BF16 = mybir.dt.bfloat16
F32R = mybir.dt.float32r

DT = BF16

B = 8
T = 4096
C = 1024
H = 64

TS = 512          # t-slice width (stage A) and q-block width (stage B)
N_CORES = 8


def build_nc(t_len: int = T, dt_c=None):
    """Build + compile the per-core Bass program for sequence length t_len."""
    if dt_c is None:
        dt_c = DT
    assert t_len % TS == 0
    n_slice = t_len // TS          # t-slices / q-blocks
    n_chunk = t_len // 128         # 128-wide k-chunks / t-tiles

    nc = bacc.Bacc(None, target_bir_lowering=False, debug=False)

    x_d = nc.dram_tensor("x", [t_len, C], F32, kind="ExternalInput")
    wq_d = nc.dram_tensor("wq", [C, H], F32, kind="ExternalInput")
    wk_d = nc.dram_tensor("wk", [C, H], F32, kind="ExternalInput")
    wv_d = nc.dram_tensor("wv", [C, H], F32, kind="ExternalInput")
    out_d = nc.dram_tensor("out", [t_len, H], F32, kind="ExternalOutput")

    with tile.TileContext(nc) as tc, ExitStack() as ctx:
        const_pool = ctx.enter_context(tc.tile_pool(name="const", bufs=1))
        res_pool = ctx.enter_context(tc.tile_pool(name="resident", bufs=1))
        xa_pool = ctx.enter_context(tc.tile_pool(name="xa", bufs=2))
        xt_pool = ctx.enter_context(tc.tile_pool(name="xt", bufs=2))
        at_pool = ctx.enter_context(tc.tile_pool(name="at", bufs=5))
        osb_pool = ctx.enter_context(tc.tile_pool(name="osb", bufs=2))
        fin_pool = ctx.enter_context(tc.tile_pool(name="fin", bufs=2))
        rec_pool = ctx.enter_context(tc.tile_pool(name="rec", bufs=2))
        ps_a = ctx.enter_context(tc.tile_pool(name="ps_a", bufs=3, space="PSUM"))
        ps_s = ctx.enter_context(tc.tile_pool(name="ps_s", bufs=2, space="PSUM"))
        ps_o = ctx.enter_context(tc.tile_pool(name="ps_o", bufs=1, space="PSUM"))

        # ---- x loads: gpsimd (SWDGE) DMAs cast fp32 -> bf16 in flight.
        # xa[s][g][p, c] = x[s*TS + 128*g + p, c]. Slices 0,1 are loaded as
        # four per-g DMAs each (fine-grained deps so the first transposes
        # start after 512 KB, not 2 MB); later slices use one DMA.
        xa_tiles: dict = {}
        xa_coarse: dict = {}

        def issue_x_load(s, gs=(0, 1, 2, 3)):
            if s not in xa_tiles:
                xa_tiles[s] = [
                    xa_pool.tile([128, C], DT, tag=f"xag{g}",
                                 name=f"xa{s}_{g}")
                    for g in range(4)
                ]
            for g in gs:
                nc.gpsimd.dma_start(
                    out=xa_tiles[s][g][:],
                    in_=x_d[s * TS + 128 * g : s * TS + 128 * (g + 1), :],
                )

        # slice 0 of x starts loading before anything else hits the Pool
        # queue (its DMA gates the first transposes)
        issue_x_load(0)

        # ---- constants ----
        identf = const_pool.tile([128, 128], F32, tag="identf")
        make_identity(nc, identf[:])
        # ident2[64+i, i] = 1 (identity content living at partitions 64:128)
        scr2 = const_pool.tile([128, H], F32, tag="scr2")
        nc.gpsimd.memset(scr2[:], 0.0)
        nc.gpsimd.affine_select(
            out=scr2[:],
            in_=scr2[:],
            compare_op=mybir.AluOpType.not_equal,
            fill=1.0,
            base=-64,
            pattern=[[-1, H]],
            channel_multiplier=1,
        )
        ident2 = const_pool.tile([128, H], dt_c, tag="ident2")
        nc.vector.tensor_copy(ident2[:], scr2[:])
        # bf16 identity for the x transposes (bf16 streams at 1 cyc/col)
        identb = const_pool.tile([128, 128], DT, tag="identb")
        nc.vector.tensor_copy(identb[:], identf[:])
        # warm the ACT exp table set (~2.7us DMA) during the initial ramp
        warm = const_pool.tile([128, 1], F32, tag="warm")
        nc.scalar.activation(
            warm[:], scr2[:, 0:1], mybir.ActivationFunctionType.Exp
        )

        # mask[x, y] = 1.0 if y >= x else 0.0 (upper-triangular window)
        ms = const_pool.tile([128, 128], F32, tag="maskscr")
        nc.gpsimd.memset(ms[:], 1.0)
        nc.gpsimd.affine_select(
            out=ms[:],
            in_=ms[:],
            compare_op=mybir.AluOpType.is_ge,
            fill=0.0,
            base=0,
            pattern=[[1, 128]],
            channel_multiplier=-1,
        )
        mask0 = const_pool.tile([128, 128], dt_c, tag="mask0")
        nc.vector.tensor_copy(mask0[:], ms[:])

        # ---- weights: gpsimd casting DMAs straight into the packed bf16
        # stationaries [Wq|Wq] and [Wk|Wv] (no fp32 staging, no DVE casts).
        # Interleaved with the first two x-slice loads on the Pool queue so
        # slice 0 starts moving first.
        wqq = const_pool.tile([128, 8, 128], dt_c, tag="wqq")
        wkv = const_pool.tile([128, 8, 128], dt_c, tag="wkv")

        def load_w(dst, half, w_d):
            nc.gpsimd.dma_start(
                out=dst[:, :, H * half : H * (half + 1)],
                in_=w_d.rearrange("(cc p) h -> p cc h", p=128),
            )

        load_w(wqq, 0, wq_d)
        load_w(wqq, 1, wq_d)
        issue_x_load(1)
        load_w(wkv, 0, wk_d)
        load_w(wkv, 1, wv_d)

        # ---- residents (bf16), ONE tile per t-slice holding Q^T | K^T |
        # V^T | V' (fewer pool tags -> fewer semaphores -> shorter
        # epilogue). The stage driver drains all of slice s before block s
        # reads it, so the coarse per-tile deps cost nothing.
        # Layout per partition: [0:512) Q^T, [512:1024) K^T (4 chunks of
        # 128), [1024:1536) V^T, [1536:1800) V' (4 groups of 66: V|1|pad).
        res = [res_pool.tile([128, 1800], dt_c, tag=f"res{s}",
                             name=f"res{s}") for s in range(n_slice)]
        qts = [r[:, 0:TS] for r in res]
        kts = [r[:, TS : 2 * TS].rearrange("p (g c) -> p g c", c=128)
               for r in res]
        vts = [r[:, 2 * TS : 3 * TS] for r in res]
        vps = [r[:, 3 * TS : 3 * TS + 264].rearrange("p (g h) -> p g h",
                                                     h=H + 2)
               for r in res]
        # (the V' ones-columns are memset inside each a_slice, keeping the
        # Pool queue clear for the early x-slice loads)

        # ---- Stage A generator: one slice = several interleavable steps.
        # Slices 0,1: PE-transpose (low latency, fine-grained x deps).
        # Slices 2+: XBAR dma_start_transpose (16x128 bf16 tiles) — the
        # transpose rides the DMA fabric, freeing ~20us of PE time; its
        # output layout is xt[p, g, cc, tau] = x^T[128cc+p, 128g+tau].
        def a_slice(s):
            xa = xa_tiles[s]
            # XBAR transposes measured SLOWER end-to-end (DMA-fabric
            # contention with the x loads); keep all transposes on the PE.
            use_xbar = False
            if use_xbar:
                xt = xt_pool.tile([128, 4, 8, 128], DT, tag="xtx",
                                  name=f"xt{s}")
                nc.sync.dma_start_transpose(xt[:], xa_coarse[s][:])
                rhs_of = lambda cc: xt[:, :, cc, :]
            else:
                xt = xt_pool.tile([128, 8, TS], DT, tag="xt", name=f"xt{s}")
                rhs_of = lambda cc: xt[:, cc, :]
            if s + 2 < n_slice:
                issue_x_load(s + 2)
            nc.gpsimd.memset(vps[s][:, :, H : H + 1], 1.0)
            if not use_xbar:
                for g in range(4):
                    pp = ps_a.tile([128, 8, 128], DT, tag="psA")
                    for cc in range(8):
                        nc.tensor.matmul(
                            pp[:, cc, :],
                            xa[g][:, 128 * cc : 128 * (cc + 1)],
                            identb[:],
                            is_transpose=True,
                            start=(cc == 0),
                            stop=(cc == 7),
                            skip_group_check=True,
                        )
                    # early slices alternate the evacuation between the ACT
                    # engine (idle until the first attention block) and DVE
                    # so the evac latency never paces the transpose stream
                    dst = xt[:, :, 128 * g : 128 * (g + 1)]
                    if s < 4 and g % 2 == 0:
                        nc.scalar.copy(dst, pp[:])
                    else:
                        nc.vector.tensor_copy(dst, pp[:])
                    yield
            else:
                yield
            # pass 1: [Wq|Wq]
            psp1 = ps_a.tile([128, TS], F32, tag="psA")
            for cc in range(4):
                nc.tensor.matmul(
                    psp1[:], wqq[:, cc, :], rhs_of(cc),
                    start=(cc == 0), stop=False,
                )
            yield
            for cc in range(4, 8):
                nc.tensor.matmul(
                    psp1[:], wqq[:, cc, :], rhs_of(cc),
                    start=False, stop=(cc == 7),
                )
            nc.vector.tensor_copy(qts[s][:], psp1[:])
            yield
            # pass 2: [Wk|Wv]
            psp2 = ps_a.tile([128, TS], F32, tag="psA")
            for cc in range(4):
                nc.tensor.matmul(
                    psp2[:], wkv[:, cc, :], rhs_of(cc),
                    start=(cc == 0), stop=False,
                )
            yield
            for cc in range(4, 8):
                nc.tensor.matmul(
                    psp2[:], wkv[:, cc, :], rhs_of(cc),
                    start=False, stop=(cc == 7),
                )
            nc.vector.tensor_copy(
                kts[s][0:64, :, :],
                psp2[0:64, :].rearrange("p (g c) -> p g c", c=128),
            )
            nc.vector.tensor_copy(vts[s][64:128, :], psp2[64:128, :])
            # duplicate K^T onto partitions 64:128: the pair's two score
            # matmuls then read disjoint SBUF partition halves (port
            # parallelism), which is what lets them pipeline tightly
            nc.sync.dma_start(kts[s][64:128, :, :], kts[s][0:64, :, :])
            yield
            # V' build: V natural [tk, 64] + ones column
            psv = ps_a.tile([128, TS], dt_c, tag="psA")
            for g in range(4):
                nc.tensor.matmul(
                    psv[:, H * g : H * (g + 1)],
                    vts[s][64:128, 128 * g : 128 * (g + 1)],
                    ident2[64:128, :],
                    is_transpose=True,
                    start=(g == 0),
                    stop=(g == 3),
                    skip_group_check=True,
                )
            nc.vector.tensor_copy(
                vps[s][:, :, 0:H],
                psv[:, 0 : 4 * H].rearrange("p (g h) -> p g h", h=H),
            )
            yield

        def a_stream():
            for s in range(n_slice):
                yield from a_slice(s)
                yield s  # slice s fully emitted

        agen = a_stream()
        a_done = [-1]

        def step_a():
            try:
                r = next(agen)
                if isinstance(r, int):
                    a_done[0] = r
            except StopIteration:
                pass

        def drain_a(upto):
            while a_done[0] < upto:
                r = next(agen)
                if isinstance(r, int):
                    a_done[0] = r

        # emit slices 0 and 1 before attention starts
        drain_a(1)

        # ---- Stage B: attention per q-block, interleaved with stage A ----
        for q in range(n_slice):
            drain_a(min(q, n_slice - 1))
            pso = ps_o.tile([H + 1, TS], F32, tag="pso")
            nj = 4 * (q + 1)

            def emit_avs(j0, ds, at):
                for u in range(2):
                    j = j0 + u
                    d = ds[u]
                    nc.tensor.matmul(
                        pso[:, d:TS],
                        vps[j // 4][:, j % 4, 0 : H + 1],
                        at[:, u, d:TS],
                        start=(j == 0),
                        stop=(j == nj - 1),
                        skip_group_check=True,
                    )

            # depth-2 software pipeline: the AV matmuls of pair k are
            # emitted after pair k+1's score matmuls and one stage-A step,
            # so pair k's exp (ACT, ~1.1us) has a ~2.3us PE-work window.
            pending = None
            for j0 in range(0, nj, 2):
                # d = first valid column of the chunk (causal): columns
                # before d are entirely masked and skipped end-to-end.
                ds = [max(0, 128 * (j0 + u) - TS * q) for u in range(2)]
                pss = ps_s.tile([128, 2, TS], F32, tag="pss")
                for u in range(2):
                    j = j0 + u
                    lo, hi = 64 * u, 64 * (u + 1)
                    nc.tensor.matmul(
                        pss[:, u, ds[u] : TS],
                        kts[j // 4][lo:hi, j % 4, :],
                        qts[q][lo:hi, ds[u] : TS],
                        start=True,
                        stop=True,
                        skip_group_check=True,
                    )
                at = at_pool.tile([128, 2, TS], dt_c)
                if ds[1] == 0:
                    nc.scalar.activation(
                        at[:], pss[:],
                        mybir.ActivationFunctionType.Exp, scale=0.125,
                    )
                else:
                    for u in range(2):
                        nc.scalar.activation(
                            at[:, u, ds[u] : TS], pss[:, u, ds[u] : TS],
                            mybir.ActivationFunctionType.Exp, scale=0.125,
                        )
                for u in range(2):
                    j = j0 + u
                    if j >= 4 * q:
                        d = ds[u]
                        # triangular window = first 128 computed columns
                        nc.vector.tensor_mul(
                            at[:, u, d : d + 128],
                            at[:, u, d : d + 128],
                            mask0[:],
                        )
                step_a()
                if pending is not None:
                    emit_avs(*pending)
                pending = (j0, ds, at)
            emit_avs(*pending)
            # O and the denominators round to bf16 before the final
            # transpose (cheaper PE transposes); normalization stays fp32
            osb = osb_pool.tile([H + 1, TS], DT)
            nc.vector.tensor_copy(osb[:], pso[:])
            # batch the 4 output transposes into one psum bank (padded to
            # H+2 per group so each bf16 group lands 4-byte aligned)
            psf = ps_a.tile([128, 4, H + 2], DT, tag="psA")
            for g in range(4):
                nc.tensor.matmul(
                    psf[:, g, 0 : H + 1],
                    osb[:, 128 * g : 128 * (g + 1)],
                    identb[0 : H + 1, 0 : H + 1],
                    is_transpose=True,
                    start=(g == 0),
                    stop=(g == 3),
                    skip_group_check=True,
                )
            rec = rec_pool.tile([128, 4, 1], F32)
            nc.vector.reciprocal(rec[:], psf[:, :, H : H + 1])
            fin = fin_pool.tile([128, 4, H], F32)
            for g in range(4):
                nc.vector.tensor_scalar_mul(
                    fin[:, g, :], psf[:, g, 0:H], rec[:, g, :]
                )
                nc.sync.dma_start(
                    out_d[q * TS + 128 * g : q * TS + 128 * (g + 1), :],
                    fin[:, g, :],
                )
        drain_a(n_slice - 1)

    nc.compile()
    return nc


_NC_CACHE: dict = {}


def _get_nc(t_len: int, dt_c=None):
    key = (t_len, dt_c or DT)
    if key not in _NC_CACHE:
        _NC_CACHE[key] = build_nc(t_len, dt_c)
    return _NC_CACHE[key]


def run_on_cores(nc, x_b: np.ndarray, wq, wk, wv):
    """Run the compiled program SPMD on the 8 cores; x_b is [B, t, C]."""
    in_maps = [
        {
            "x": np.ascontiguousarray(x_b[b]),
            "wq": np.ascontiguousarray(wq),
            "wk": np.ascontiguousarray(wk),
            "wv": np.ascontiguousarray(wv),
        }
        for b in range(x_b.shape[0])
    ]
    res = run_bass_kernel_spmd(nc, in_maps, list(range(len(in_maps))))
    return np.stack([res.results[b]["out"] for b in range(x_b.shape[0])])


def kernel(x, Wq, Wk, Wv):
    x = np.asarray(x, dtype=np.float32)
    Wq = np.asarray(Wq, dtype=np.float32)
    Wk = np.asarray(Wk, dtype=np.float32)
    Wv = np.asarray(Wv, dtype=np.float32)
    assert x.shape == (B, T, C), x.shape
    nc = _get_nc(T)
    return run_on_cores(nc, x, Wq, Wk, Wv)
